# revision 21
# baseline (speedup 1.0000x reference)
"""Trainium2 Bass kernel for GQA attention prefill with KV cache.

Problem: B=2, S=1024, DIM=4096, 32 q-heads / 8 kv-heads (GQA 4:1), HEAD_DIM=128,
CACHE_LEN=1024 (KV_LEN=2048), RoPE on q/k, causal mask over the new block.

Sharding: tensor-parallel over heads across 8 cores. Each core gets 4 q-heads /
1 kv-head: wq/wk/wv column shards, wo row shard, KV-cache shard. Each core
computes a full-shape partial output (row-parallel wo); host sums the partials.

Fast causal path: 4-stage cross-batch software pipeline emitted with
cost-weighted generator interleaving so the PE never drains:
  stage 1: proj(b0)                 stage 2: attn(b0) || proj(b1)
  stage 3: oproj(b0) || attn(b1)    stage 4: oproj(b1)
Per-pair attention chain (sc matmul -> exp on ACT -> av matmul) is software
pipelined one pair deep; softmax-sum accumulation runs on the idle GpSimd
engine; the denominator reciprocal uses the fast approx DVE op; oproj drains
split ~80/20 between DVE and ACT. V is projected transposed (wide matmuls)
and flipped to natural layout with PE transposes.
"""
import math
import numpy as np
import ml_dtypes

import concourse.bacc as bacc
import concourse.mybir as mybir
import concourse.tile as tile
from concourse.bass_utils import run_bass_kernel_spmd

BF16 = ml_dtypes.bfloat16
F32 = mybir.dt.float32
F32R = mybir.dt.float32r
BF = mybir.dt.bfloat16

B, S, DIM = 2, 1024, 4096
N_HEADS, N_KV = 32, 8
HD = 128
QH = N_HEADS // 8          # q heads per core = 4
CACHE = 1024
KV = CACHE + S             # 2048
KCH = DIM // 128           # 32 contraction chunks
NCH = KV // 128            # 16 kv chunks
SBW = 512                  # s-block width
NSB = S // SBW             # 2
NKG = 8                    # xT DMA chunk groups (4 ko each)
SCALE = 1.0 / math.sqrt(HD)

_CACHE = {}


def _pairs_for_mode(mode):
    """Per s-block list of (mc, n_off, tri, masked). First pair of each block
    must be full-width (n_off == 0)."""
    pairs = {sb: [] for sb in range(NSB)}
    for sb in range(NSB):
        sb_start = sb * SBW
        for mc in range(NCH):
            if mode == "causal" and mc >= CACHE // 128:
                s0 = (mc - CACHE // 128) * 128   # first query that sees this chunk
                if s0 >= sb_start + SBW:
                    continue                      # chunk entirely in the future
                n_off = max(0, s0 - sb_start)
                tri = s0 >= sb_start              # diagonal strip inside block
                pairs[sb].append((mc, n_off, tri, mode == "generic"))
            else:
                pairs[sb].append((mc, 0, False, mode == "generic"))
        # ensure first pair full width
        assert pairs[sb][0][1] == 0
    return pairs


def build_nc(mode, reps=1, phases=('proj', 'attn', 'oproj')):
    if mode != "causal":
        return _build_nc_ref(mode, reps, phases)
    nc = bacc.Bacc("TRN2", target_bir_lowering=False, debug=False, num_devices=8)

    # xt chunked for streaming: [b, sb, kg, 128, 4ko, 512s]
    xt = nc.dram_tensor("xt", [B, NSB, NKG, 128, 4, SBW], mybir.dt.bfloat16,
                        kind="ExternalInput")
    wq = nc.dram_tensor("wq", [128, QH, KCH, HD], mybir.dt.bfloat16, kind="ExternalInput")
    wk = nc.dram_tensor("wk", [128, KCH, HD], mybir.dt.bfloat16, kind="ExternalInput")
    wv = nc.dram_tensor("wv", [128, KCH, HD], mybir.dt.bfloat16, kind="ExternalInput")
    wo = nc.dram_tensor("wo", [128, QH, DIM], mybir.dt.bfloat16, kind="ExternalInput")
    kc = nc.dram_tensor("kc", [B, 128, CACHE], mybir.dt.bfloat16, kind="ExternalInput")
    vc = nc.dram_tensor("vc", [B, 128, CACHE], mybir.dt.bfloat16, kind="ExternalInput")
    cc = nc.dram_tensor("cc", [128, S], F32, kind="ExternalInput")
    ss = nc.dram_tensor("ss", [128, S], F32, kind="ExternalInput")
    tri = nc.dram_tensor("tri", [128, 128], F32, kind="ExternalInput")
    ident = nc.dram_tensor("ident", [128, 128], mybir.dt.bfloat16, kind="ExternalInput")
    out = nc.dram_tensor("o", [B * S, DIM], mybir.dt.bfloat16, kind="ExternalOutput")

    pairs = _pairs_for_mode(mode)

    with tile.TileContext(nc) as tc:
        with (
            tc.tile_pool(name="const", bufs=1) as const,
            tc.tile_pool(name="big", bufs=2) as big,
            tc.tile_pool(name="work", bufs=2) as work,
            tc.tile_pool(name="ps_mm", bufs=2, space="PSUM") as ps_mm,
            tc.tile_pool(name="ps_sc", bufs=3, space="PSUM") as ps_sc,
            tc.tile_pool(name="ps_av", bufs=2, space="PSUM") as ps_av,
        ):
          from contextlib import nullcontext
          _hints = (mybir.EngineType.PE, mybir.EngineType.DVE,
                    mybir.EngineType.Activation, mybir.EngineType.Pool,
                    mybir.EngineType.SP)
          with (tc.For_i(0, reps, 1, hint_engines=_hints) if reps > 1
                else nullcontext()):
            # ---- constants ----
            wk_sb = const.tile([128, KCH, HD], BF)
            nc.sync.dma_start(wk_sb, wk.ap())
            wv_sb = const.tile([128, KCH, HD], BF)
            nc.sync.dma_start(wv_sb, wv.ap())
            cc_sb = const.tile([128, S], F32)
            nc.sync.dma_start(cc_sb, cc.ap())
            ss_sb = const.tile([128, S], F32)
            nc.sync.dma_start(ss_sb, ss.ap())
            tri_sb = const.tile([128, 128], F32)
            nc.sync.dma_start(tri_sb, tri.ap())
            id_sb = const.tile([128, 128], BF)
            nc.sync.dma_start(id_sb, ident.ap())
            ones_bf = const.tile([128, 1], BF)
            nc.vector.memset(ones_bf, 1.0)
            ones_k1 = const.tile([1, 128], BF)
            nc.vector.memset(ones_k1, 1.0)

            def rope_store(ps, dest, scol):
                """dest[0:64] = ps[0:64]*cc - ps[64:]*ss ; dest[64:] = ps[0:64]*ss + ps[64:]*cc
                ps: [128, SBW] psum f32; dest: [128, SBW] sbuf bf16 slice; scol: s offset.
                Every tensor_tensor keeps equal input base partitions (walrus
                birverifier requirement); cross terms write to the other half."""
                cslc = cc_sb[:, scol:scol + SBW]
                sslc = ss_sb[:, scol:scol + SBW]
                t1 = work.tile([128, SBW], F32, name="rope_t1")
                nc.vector.tensor_mul(t1, ps, cslc)
                t2 = work.tile([128, SBW], F32, name="rope_t2")
                nc.vector.tensor_mul(t2[0:64, :], ps[64:128, :], sslc[64:128, :])
                nc.vector.tensor_mul(t2[64:128, :], ps[0:64, :], sslc[0:64, :])
                nc.vector.tensor_sub(dest[0:64, :], t1[0:64, :], t2[0:64, :])
                nc.vector.tensor_add(dest[64:128, :], t2[64:128, :], t1[64:128, :])

            # per-batch persistent tiles
            state = {}

            def batch_tiles(b):
                st = {}
                st['xt'] = {}
                st['kT'] = big.tile([128, KV], BF, name="kT")
                st['v'] = big.tile([128, NCH * 128], BF, name="v_sb")
                st['qT'] = big.tile([128, QH, S], BF, name="qT")
                st['outT'] = big.tile([128, QH, S], BF, name="outT")
                return st

            MMNS = 230      # ~ns per 512-wide matmul incl overhead

            def xsl(st, sb, ko):
                return st['xt'][(sb, ko // 4)][:, ko % 4, :]

            def kv_stream(b):
                """Setup + K and V projections. MUST be first-advanced before
                q_stream(b) (creates state[b])."""
                st = state[b] = batch_tiles(b)
                # x chunks first (feed Kproj startup), kv cache after
                for kg in range(NKG):
                    for sb in range(NSB):
                        t = big.tile([128, 4, SBW], BF, name="xt_t", bufs=20)
                        nc.sync.dma_start(t, xt.ap()[b, sb, kg])
                        st['xt'][(sb, kg)] = t
                nc.sync.dma_start(st['kT'][:, 0:CACHE], kc.ap()[b])
                nc.sync.dma_start(st['v'][:, 0:CACHE], vc.ap()[b])
                yield 0

                # ---- K projection + RoPE ----
                for sb in range(NSB):
                    ps = ps_mm.tile([128, SBW], F32, name="mm_ps", bufs=1)
                    for ko in range(KCH):
                        nc.tensor.matmul(ps, wk_sb[:, ko, :], xsl(st, sb, ko),
                                         start=(ko == 0), stop=(ko == KCH - 1))
                        if ko % 2 == 1:
                            yield MMNS * 2
                    rope_store(ps, st['kT'][:, CACHE + sb * SBW: CACHE + (sb + 1) * SBW],
                               sb * SBW)
                    yield 0

                # ---- V projection (transposed, then PE-transpose to natural) ----
                for sb in range(NSB):
                    ps = ps_mm.tile([128, SBW], F32, name="mm_ps", bufs=1)
                    for ko in range(KCH):
                        nc.tensor.matmul(ps, wv_sb[:, ko, :], xsl(st, sb, ko),
                                         start=(ko == 0), stop=(ko == KCH - 1))
                        if ko % 2 == 1:
                            yield MMNS * 2
                    vt = work.tile([128, SBW], BF, name="vT_sb")
                    nc.vector.tensor_copy(vt, ps)
                    yield 0
                    pst = ps_mm.tile([128, SBW], BF, name="mm_ps", bufs=1)
                    for j in range(4):
                        nc.tensor.transpose(
                            pst[:, j * 128:(j + 1) * 128],
                            vt[:, j * 128:(j + 1) * 128], id_sb)
                    yield MMNS
                    nc.vector.tensor_copy(
                        st['v'][:, CACHE + sb * SBW: CACHE + (sb + 1) * SBW], pst)
                    yield 0

            def q_stream(b):
                """Q projection + RoPE; advance only after kv_stream(b) started."""
                st = state[b]
                for h in range(QH):
                    wq_p = []
                    for pc in range(4):
                        t = work.tile([128, 8, HD], BF, name="wq_t", bufs=5)
                        nc.sync.dma_start(t, wq.ap()[:, h, pc * 8:(pc + 1) * 8, :])
                        wq_p.append(t)
                    for sb in range(NSB):
                        ps = ps_mm.tile([128, SBW], F32, name="mm_ps2")
                        for ko in range(KCH):
                            nc.tensor.matmul(ps, wq_p[ko // 8][:, ko % 8, :],
                                             xsl(st, sb, ko),
                                             start=(ko == 0), stop=(ko == KCH - 1))
                            if ko % 2 == 1:
                                yield MMNS * 2
                        rope_store(ps, st['qT'][:, h, sb * SBW:(sb + 1) * SBW],
                                   sb * SBW)
                        yield 0

            def merged(*gens):
                """Time-balanced interleave; first generator is advanced first
                (ordering matters when later gens read its setup state)."""
                n = len(gens)
                acc = [0.0] * n
                cur = [next(g, None) for g in gens]
                while any(c is not None for c in cur):
                    i = min((a for a in range(n) if cur[a] is not None),
                            key=lambda a: acc[a])
                    acc[i] += cur[i]
                    yield cur[i]
                    cur[i] = next(gens[i], None)

            def proj_stream(b):
                """kv and q interleaved (startup overlap within a batch)."""
                return merged(kv_stream(b), q_stream(b))

            def attn_stream(b):
                st = state[b]
                ep = None       # pending epilogue closure chain
                for sb in range(NSB):
                    plist = pairs[sb]
                    for h in range(QH):
                        # two parallel softmax-sum chains: DVE gets every 3rd
                        # pair, GpSimd the rest (GpSimd op ~2x slower)
                        acc_a = work.tile([128, SBW], BF, name="acc_a", bufs=2)
                        acc_b = work.tile([128, SBW], BF, name="acc_b", bufs=2)
                        na = nb_ = 0
                        av = ps_av.tile([128, SBW], F32, name="av_ps")
                        pend = []
                        av_n = 0
                        for pi, (mc, n_off, use_tri, _) in enumerate(plist):
                            sc = ps_sc.tile([128, SBW], F32, name="sc_ps")
                            nc.tensor.matmul(
                                sc[:, n_off:], st['kT'][:, mc * 128:(mc + 1) * 128],
                                st['qT'][:, h, sb * SBW + n_off:(sb + 1) * SBW],
                                start=True, stop=True)
                            if use_tri:
                                nc.vector.tensor_add(
                                    sc[:, n_off:n_off + 128],
                                    sc[:, n_off:n_off + 128], tri_sb)
                            ex = work.tile([128, SBW], BF, name="ex", bufs=5)
                            nc.scalar.activation(ex[:, n_off:], sc[:, n_off:],
                                                 mybir.ActivationFunctionType.Exp,
                                                 scale=SCALE)
                            if pi % 3 == 0:
                                if na == 0:
                                    nc.vector.tensor_copy(acc_a, ex)
                                else:
                                    nc.vector.tensor_add(acc_a[:, n_off:],
                                                         acc_a[:, n_off:],
                                                         ex[:, n_off:])
                                na += 1
                            else:
                                if nb_ == 0:
                                    nc.gpsimd.tensor_copy(acc_b, ex)
                                else:
                                    nc.gpsimd.tensor_add(acc_b[:, n_off:],
                                                         acc_b[:, n_off:],
                                                         ex[:, n_off:])
                                nb_ += 1
                            pend.append((mc, n_off, ex))
                            if len(pend) > 2:
                                pmc, poff, pex = pend.pop(0)
                                nc.tensor.matmul(av[:, poff:],
                                                 st['v'][:, pmc * 128:(pmc + 1) * 128],
                                                 pex[:, poff:],
                                                 start=(av_n == 0), stop=False)
                                av_n += 1
                            if ep is not None:
                                ep = ep()   # run one lagged epilogue step
                            yield MMNS * 2
                        # flush remaining avs; denominator lagged into next block
                        for j, (pmc, poff, pex) in enumerate(pend):
                            nc.tensor.matmul(av[:, poff:],
                                             st['v'][:, pmc * 128:(pmc + 1) * 128],
                                             pex[:, poff:],
                                             start=(av_n == 0),
                                             stop=(j == len(pend) - 1))
                            av_n += 1
                            if ep is not None:
                                ep = ep()
                            yield MMNS
                        while ep is not None:
                            ep = ep()

                        def make_ep(acc_a=acc_a, acc_b=acc_b, av=av, h=h, sb=sb):
                            def e0():
                                den = ps_sc.tile([128, SBW], F32, name="sc_ps")
                                nc.tensor.matmul(den[0:1, :], ones_bf, acc_a,
                                                 start=True, stop=False)
                                nc.tensor.matmul(den[0:1, :], ones_bf, acc_b,
                                                 start=False, stop=True)

                                def e1():
                                    rc = work.tile([1, SBW], F32, name="rc", bufs=2)
                                    nc.vector.reciprocal_approx_fast(rc, den[0:1, :])
                                    rcb = work.tile([1, SBW], BF, name="rcb", bufs=2)
                                    nc.scalar.copy(rcb, rc)

                                    def e2():
                                        rb = ps_sc.tile([128, SBW], F32, name="sc_ps")
                                        nc.tensor.matmul(rb, ones_k1, rcb,
                                                         start=True, stop=True)
                                        rbs = work.tile([128, SBW], F32, name="rbs",
                                                        bufs=2)
                                        nc.vector.tensor_copy(rbs, rb)

                                        def e3():
                                            nc.vector.tensor_mul(
                                                st['outT'][:, h,
                                                           sb * SBW:(sb + 1) * SBW],
                                                av, rbs)
                                            return None
                                        return e3
                                    return e2
                                return e1
                            return e0
                        ep = make_ep()
                while ep is not None:
                    ep = ep()
                yield 0

            def oproj_stream(b, act_mod=5, extra_pools=False):
                st = state[b]
                ui = 0
                pool_cycle = ([(ps_mm, "mm_ps2"), (ps_sc, "sc_ps"),
                               (ps_av, "av_ps")] if extra_pools
                              else [(ps_mm, "mm_ps2")])
                for nb in range(DIM // SBW):
                    wo_t = work.tile([128, QH, SBW], BF, name="wo_t", bufs=2)
                    nc.sync.dma_start(wo_t[:, 0:2], wo.ap()[:, 0:2, nb * SBW:(nb + 1) * SBW])
                    nc.sync.dma_start(wo_t[:, 2:4], wo.ap()[:, 2:4, nb * SBW:(nb + 1) * SBW])
                    for stt in range(S // 128):
                        pool, tag = pool_cycle[ui % len(pool_cycle)]
                        ps = pool.tile([128, SBW], F32, name=tag)
                        for h in range(QH):
                            nc.tensor.matmul(ps, st['outT'][:, h, stt * 128:(stt + 1) * 128],
                                             wo_t[:, h, :],
                                             start=(h == 0), stop=(h == QH - 1))
                        os_t = work.tile([128, SBW], BF, name="os_t", bufs=4)
                        if ui % act_mod == act_mod - 1:
                            nc.scalar.copy(os_t, ps)
                        else:
                            nc.vector.tensor_copy(os_t, ps)
                        nc.sync.dma_start(
                            out.ap()[b * S + stt * 128:b * S + (stt + 1) * 128,
                                     nb * SBW:(nb + 1) * SBW], os_t)
                        ui += 1
                        yield MMNS * 4

            def drain(g):
                for _ in g:
                    pass

            def merge2(ga, gb):
                ta = tb = 0.0
                ca = next(ga, None)
                cb = next(gb, None)
                while ca is not None or cb is not None:
                    if cb is None or (ca is not None and ta <= tb):
                        ta += ca
                        ca = next(ga, None)
                    else:
                        tb += cb
                        cb = next(gb, None)

            drain(proj_stream(0))                        # stage 1
            merge2(attn_stream(0), proj_stream(1))       # stage 2
            merge2(oproj_stream(0, 5), attn_stream(1))   # stage 3
            drain(oproj_stream(1, 2, extra_pools=True))  # stage 4

    nc.compile()
    return nc


def _build_nc_ref(mode, reps=1, phases=('proj', 'attn', 'oproj')):
    nc = bacc.Bacc("TRN2", target_bir_lowering=False, debug=False, num_devices=8)

    xt = nc.dram_tensor("xt", [128, KCH, B * S], mybir.dt.bfloat16, kind="ExternalInput")
    wq = nc.dram_tensor("wq", [128, QH, KCH, HD], mybir.dt.bfloat16, kind="ExternalInput")
    wk = nc.dram_tensor("wk", [128, KCH, HD], mybir.dt.bfloat16, kind="ExternalInput")
    wv = nc.dram_tensor("wv", [128, KCH, HD], mybir.dt.bfloat16, kind="ExternalInput")
    wo = nc.dram_tensor("wo", [128, QH, DIM], mybir.dt.bfloat16, kind="ExternalInput")
    kc = nc.dram_tensor("kc", [B, 128, CACHE], mybir.dt.bfloat16, kind="ExternalInput")
    vc = nc.dram_tensor("vc", [B, 128, CACHE], mybir.dt.bfloat16, kind="ExternalInput")
    cc = nc.dram_tensor("cc", [128, S], F32, kind="ExternalInput")
    ss = nc.dram_tensor("ss", [128, S], F32, kind="ExternalInput")
    tri = nc.dram_tensor("tri", [128, 128], F32, kind="ExternalInput")
    if mode == "generic":
        mt = nc.dram_tensor("maskt", [128, NCH, S], mybir.dt.bfloat16, kind="ExternalInput")
    out = nc.dram_tensor("o", [B * S, DIM], mybir.dt.bfloat16, kind="ExternalOutput")

    pairs = _pairs_for_mode(mode)

    with tile.TileContext(nc) as tc:
        with (
            tc.tile_pool(name="const", bufs=1) as const,
            tc.tile_pool(name="xpool", bufs=1) as xpool,
            tc.tile_pool(name="perb", bufs=2) as perb,
            tc.tile_pool(name="work", bufs=2) as work,
            tc.tile_pool(name="ps_mm", bufs=3, space="PSUM") as ps_mm,
            tc.tile_pool(name="ps_av", bufs=2, space="PSUM") as ps_av,
            tc.tile_pool(name="ps_sm", bufs=1, space="PSUM") as ps_sm,
        ):
          from contextlib import nullcontext
          with (tc.For_i(0, reps, 1) if reps > 1 else nullcontext()):
            # ---- constants ----
            wk_sb = const.tile([128, KCH, HD], BF)
            nc.sync.dma_start(wk_sb, wk.ap())
            wv_sb = const.tile([128, KCH, HD], BF)
            nc.sync.dma_start(wv_sb, wv.ap())
            cc_sb = const.tile([128, S], F32)
            nc.sync.dma_start(cc_sb, cc.ap())
            ss_sb = const.tile([128, S], F32)
            nc.sync.dma_start(ss_sb, ss.ap())
            tri_sb = const.tile([128, 128], F32)
            nc.sync.dma_start(tri_sb, tri.ap())
            ones_bf = const.tile([128, 1], BF)
            nc.vector.memset(ones_bf, 1.0)
            ones_k1 = const.tile([1, 128], BF)
            nc.vector.memset(ones_k1, 1.0)

            def rope_store(ps, dest, scol):
                cslc = cc_sb[:, scol:scol + SBW]
                sslc = ss_sb[:, scol:scol + SBW]
                t1 = work.tile([128, SBW], F32, name="rope_t1")
                nc.vector.tensor_mul(t1, ps, cslc)
                t2 = work.tile([128, SBW], F32, name="rope_t2")
                nc.vector.tensor_mul(t2[0:64, :], ps[64:128, :], sslc[64:128, :])
                nc.vector.tensor_mul(t2[64:128, :], ps[0:64, :], sslc[0:64, :])
                nc.vector.tensor_sub(dest[0:64, :], t1[0:64, :], t2[0:64, :])
                nc.vector.tensor_add(dest[64:128, :], t2[64:128, :], t1[64:128, :])

            for b in range(B):
                xT = xpool.tile([128, KCH, S], BF, name="xT")
                for g in range(4):
                    for hh in range(2):
                        nc.sync.dma_start(
                            xT[:, hh * 16:(hh + 1) * 16, g * 256:(g + 1) * 256],
                            xt.ap()[:, hh * 16:(hh + 1) * 16,
                                    b * S + g * 256: b * S + (g + 1) * 256])
                kT = perb.tile([128, KV], BF, name="kT")
                nc.sync.dma_start(kT[:, 0:CACHE], kc.ap()[b])
                v_sb = perb.tile([128, NCH, 128], BF, name="v_sb")
                nc.sync.dma_start(
                    v_sb[:, 0:CACHE // 128, :],
                    vc.ap()[b].rearrange("p (c j) -> p c j", j=128))

                for sb in (range(NSB) if 'proj' in phases else []):
                    ps = ps_mm.tile([128, SBW], F32, name="mm_ps")
                    for ko in range(KCH):
                        nc.tensor.matmul(ps, wk_sb[:, ko, :],
                                         xT[:, ko, sb * SBW:(sb + 1) * SBW],
                                         start=(ko == 0), stop=(ko == KCH - 1))
                    rope_store(ps, kT[:, CACHE + sb * SBW: CACHE + (sb + 1) * SBW],
                               sb * SBW)

                qT = perb.tile([128, QH, S], BF, name="qT")
                for h in (range(QH) if 'proj' in phases else []):
                    wq_blk = work.tile([128, KCH, HD], BF, name="wq_blk")
                    nc.sync.dma_start(wq_blk, wq.ap()[:, h])
                    for sb in range(NSB):
                        ps = ps_mm.tile([128, SBW], F32, name="mm_ps")
                        for ko in range(KCH):
                            nc.tensor.matmul(ps, wq_blk[:, ko, :],
                                             xT[:, ko, sb * SBW:(sb + 1) * SBW],
                                             start=(ko == 0), stop=(ko == KCH - 1))
                        rope_store(ps, qT[:, h, sb * SBW:(sb + 1) * SBW], sb * SBW)

                for mt_i in (range(S // 128) if 'proj' in phases else []):
                    ps = ps_mm.tile([128, SBW], F32, name="mm_ps")
                    for ko in range(KCH):
                        nc.tensor.matmul(
                            ps[:, 0:128],
                            xT[:, ko, mt_i * 128:(mt_i + 1) * 128],
                            wv_sb[:, ko, :],
                            start=(ko == 0), stop=(ko == KCH - 1))
                    if mt_i % 2 == 0:
                        nc.scalar.copy(v_sb[:, CACHE // 128 + mt_i, :], ps[:, 0:128])
                    else:
                        nc.vector.tensor_copy(v_sb[:, CACHE // 128 + mt_i, :], ps[:, 0:128])

                outT = None
                if 'attn' in phases or 'oproj' in phases:
                    outT = perb.tile([128, QH, S], BF, name="outT", bufs=1)
                    if 'attn' not in phases:
                        nc.vector.memset(outT, 0.0)
                for h in (range(QH) if 'attn' in phases else []):
                    for sb in range(NSB):
                        av = ps_av.tile([128, SBW], F32, name="av_ps")
                        acc = work.tile([128, SBW], BF, name="acc", bufs=3)
                        plist = pairs[sb]
                        for pi, (mc, n_off, use_tri, use_mask) in enumerate(plist):
                            w = SBW - n_off
                            sc = ps_mm.tile([128, SBW], F32, name="mm_ps")
                            nc.tensor.matmul(
                                sc[:, n_off:], kT[:, mc * 128:(mc + 1) * 128],
                                qT[:, h, sb * SBW + n_off:(sb + 1) * SBW],
                                start=True, stop=True)
                            if use_tri:
                                nc.vector.tensor_add(
                                    sc[:, n_off:n_off + 128],
                                    sc[:, n_off:n_off + 128], tri_sb)
                            if use_mask:
                                mblk = work.tile([128, SBW], BF, name="mblk", bufs=3)
                                nc.sync.dma_start(
                                    mblk, mt.ap()[:, mc, sb * SBW:(sb + 1) * SBW])
                                nc.vector.tensor_add(sc[:, n_off:], sc[:, n_off:],
                                                     mblk[:, n_off:])
                            ex = work.tile([128, SBW], BF, name="ex", bufs=6)
                            nc.scalar.activation(ex[:, n_off:], sc[:, n_off:],
                                                 mybir.ActivationFunctionType.Exp,
                                                 scale=SCALE)
                            nc.tensor.matmul(av[:, n_off:], v_sb[:, mc, :],
                                             ex[:, n_off:],
                                             start=(pi == 0), stop=(pi == len(plist) - 1))
                            if pi == 0:
                                nc.vector.tensor_copy(acc, ex)
                            else:
                                nc.vector.tensor_add(acc[:, n_off:], acc[:, n_off:],
                                                     ex[:, n_off:])
                        den = ps_sm.tile([1, SBW], F32, name="den_ps", bufs=2)
                        nc.tensor.matmul(den, ones_bf, acc, start=True, stop=True)
                        rc = work.tile([1, SBW], BF, name="rc")
                        with nc.allow_low_precision(reason="1/denom in bf16 feeds bf16 matmuls"):
                            nc.vector.reciprocal(rc, den)
                        rb = ps_sm.tile([128, SBW], F32, name="rb_ps")
                        nc.tensor.matmul(rb, ones_k1, rc, start=True, stop=True)
                        rbs = work.tile([128, SBW], F32, name="rbs")
                        nc.scalar.copy(rbs, rb)
                        nc.vector.tensor_mul(outT[:, h, sb * SBW:(sb + 1) * SBW],
                                             av, rbs)

                for nb in (range(DIM // SBW) if 'oproj' in phases else []):
                    wo_blk = work.tile([128, QH, SBW], BF, name="wo_blk", bufs=3)
                    nc.sync.dma_start(wo_blk[:, 0:2], wo.ap()[:, 0:2, nb * SBW:(nb + 1) * SBW])
                    nc.sync.dma_start(wo_blk[:, 2:4], wo.ap()[:, 2:4, nb * SBW:(nb + 1) * SBW])
                    for st in range(S // 128):
                        ps = ps_mm.tile([128, SBW], F32, name="mm_ps")
                        for h in range(QH):
                            nc.tensor.matmul(ps, outT[:, h, st * 128:(st + 1) * 128],
                                             wo_blk[:, h, :],
                                             start=(h == 0), stop=(h == QH - 1))
                        os_t = work.tile([128, SBW], BF, name="os_t", bufs=6)
                        if st % 2 == 0:
                            nc.scalar.copy(os_t, ps)
                        else:
                            nc.vector.tensor_copy(os_t, ps)
                        nc.sync.dma_start(
                            out.ap()[b * S + st * 128:b * S + (st + 1) * 128,
                                     nb * SBW:(nb + 1) * SBW], os_t)

    nc.compile()
    return nc


def _detect_mode(mask):
    m = np.asarray(mask).reshape(S, KV)
    if not np.all(m[:, :CACHE] == 0):
        return "generic"
    new = m[:, CACHE:]
    neg = np.min(new)
    if neg == 0:
        return "full" if np.all(new == 0) else "generic"
    causal = np.triu(np.full((S, S), neg, dtype=np.float32), k=1)
    if np.array_equal(new, causal):
        return "causal"
    return "generic"


def prepare_inputs(x, freqs_cos, freqs_sin, mask, cache_k, cache_v, wq, wk, wv, wo,
                   mode):
    """Build the 8 per-core in_maps (numpy, host-side sharding + layout)."""
    perm = np.concatenate([np.arange(0, HD, 2), np.arange(1, HD, 2)])
    x2 = np.asarray(x, np.float32).reshape(B * S, DIM)
    xt_flat = np.ascontiguousarray(
        x2.reshape(B * S, KCH, 128).transpose(2, 1, 0)).astype(BF16)  # [128,KCH,BS]
    if mode == "causal":
        # [b, sb, kg, 128, 4, SBW]
        xt = np.ascontiguousarray(
            xt_flat.reshape(128, NKG, 4, B, NSB, SBW).transpose(3, 4, 1, 0, 2, 5))
    else:
        xt = xt_flat
    cos = np.asarray(freqs_cos, np.float32)
    sin = np.asarray(freqs_sin, np.float32)
    cc = np.ascontiguousarray(np.vstack([cos.T, cos.T]))  # [128, S]
    ss = np.ascontiguousarray(np.vstack([sin.T, sin.T]))
    m0 = np.asarray(mask, np.float32).reshape(S, KV)
    inv_scale = 1.0 / SCALE
    if mode == "causal":
        tri_np = np.ascontiguousarray(m0[0:128, CACHE:CACHE + 128].T) * inv_scale
    else:
        tri_np = np.zeros((128, 128), np.float32)
    ident_np = np.eye(128, dtype=BF16)
    maskt = None
    if mode == "generic":
        maskt = np.ascontiguousarray(
            (m0.T * inv_scale).reshape(NCH, 128, S).transpose(1, 0, 2)).astype(BF16)

    wq_f = np.asarray(wq, np.float32)
    wk_f = np.asarray(wk, np.float32)
    wv_f = np.asarray(wv, np.float32)
    wo_f = np.asarray(wo, np.float32)
    ck = np.asarray(cache_k, np.float32)
    cv = np.asarray(cache_v, np.float32)

    in_maps = []
    for c in range(8):
        wq_c = wq_f[:, c * 512:(c + 1) * 512].reshape(DIM, QH, HD)[:, :, perm]
        wq_r = np.ascontiguousarray(
            wq_c.reshape(KCH, 128, QH, HD).transpose(1, 2, 0, 3)).astype(BF16)
        wk_c = wk_f[:, c * 128:(c + 1) * 128][:, perm]
        wk_r = np.ascontiguousarray(
            wk_c.reshape(KCH, 128, HD).transpose(1, 0, 2)).astype(BF16)
        wv_c = wv_f[:, c * 128:(c + 1) * 128]
        wv_r = np.ascontiguousarray(
            wv_c.reshape(KCH, 128, HD).transpose(1, 0, 2)).astype(BF16)
        wo_c = wo_f[c * 512:(c + 1) * 512, :]
        wo_r = np.ascontiguousarray(
            wo_c.reshape(QH, 128, DIM).transpose(1, 0, 2)).astype(BF16)
        kc_r = np.ascontiguousarray(
            ck[:, c][:, :, perm].transpose(0, 2, 1)).astype(BF16)  # [B,128,CACHE]
        vc_r = np.ascontiguousarray(
            cv[:, c].reshape(B, CACHE // 128, 128, HD).transpose(0, 2, 1, 3)
            .reshape(B, 128, CACHE)).astype(BF16)
        im = {"xt": xt, "wq": wq_r, "wk": wk_r, "wv": wv_r, "wo": wo_r,
              "kc": kc_r, "vc": vc_r, "cc": cc, "ss": ss, "tri": tri_np}
        if mode == "causal":
            im["ident"] = ident_np
        if maskt is not None:
            im["maskt"] = maskt
        in_maps.append(im)
    return in_maps


def kernel(x, freqs_cos, freqs_sin, mask, cache_k, cache_v, wq, wk, wv, wo):
    mode = _detect_mode(mask)
    in_maps = prepare_inputs(x, freqs_cos, freqs_sin, mask, cache_k, cache_v,
                             wq, wk, wv, wo, mode)
    if mode not in _CACHE:
        _CACHE[mode] = build_nc(mode)
    nc = _CACHE[mode]
    res = run_bass_kernel_spmd(nc, in_maps, core_ids=list(range(8)))
    total = res.results[0]["o"].astype(np.float32)
    for c in range(1, 8):
        total += res.results[c]["o"].astype(np.float32)
    return total.reshape(B, S, DIM)


# revision 24
# speedup vs baseline: 1.0267x; 1.0267x over previous
"""Trainium2 Bass kernel for GQA attention prefill with KV cache.

Problem: B=2, S=1024, DIM=4096, 32 q-heads / 8 kv-heads (GQA 4:1), HEAD_DIM=128,
CACHE_LEN=1024 (KV_LEN=2048), RoPE on q/k, causal mask over the new block.

Sharding: tensor-parallel over heads across 8 cores. Each core gets 4 q-heads /
1 kv-head: wq/wk/wv column shards, wo row shard, KV-cache shard. Each core
computes a full-shape partial output (row-parallel wo); host sums the partials.

Fast causal path: 4-stage cross-batch software pipeline emitted with
cost-weighted generator interleaving so the PE never drains:
  stage 1: proj(b0)                 stage 2: attn(b0) || proj(b1)
  stage 3: oproj(b0) || attn(b1)    stage 4: oproj(b1)
Per-pair attention chain (sc matmul -> exp on ACT -> av matmul) is software
pipelined one pair deep; softmax-sum accumulation runs on the idle GpSimd
engine; the denominator reciprocal uses the fast approx DVE op; oproj drains
split ~80/20 between DVE and ACT. V is projected transposed (wide matmuls)
and flipped to natural layout with PE transposes.
"""
import math
import numpy as np
import ml_dtypes

import concourse.bacc as bacc
import concourse.mybir as mybir
import concourse.tile as tile
from concourse.bass_utils import run_bass_kernel_spmd

BF16 = ml_dtypes.bfloat16
F32 = mybir.dt.float32
F32R = mybir.dt.float32r
BF = mybir.dt.bfloat16

B, S, DIM = 2, 1024, 4096
N_HEADS, N_KV = 32, 8
HD = 128
QH = N_HEADS // 8          # q heads per core = 4
CACHE = 1024
KV = CACHE + S             # 2048
KCH = DIM // 128           # 32 contraction chunks
NCH = KV // 128            # 16 kv chunks
SBW = 512                  # s-block width
NSB = S // SBW             # 2
NKG = 4                    # xT DMA chunk groups (8 ko each)
SCALE = 1.0 / math.sqrt(HD)

_CACHE = {}


def _pairs_for_mode(mode):
    """Per s-block list of (mc, n_off, tri, masked). First pair of each block
    must be full-width (n_off == 0)."""
    pairs = {sb: [] for sb in range(NSB)}
    for sb in range(NSB):
        sb_start = sb * SBW
        for mc in range(NCH):
            if mode == "causal" and mc >= CACHE // 128:
                s0 = (mc - CACHE // 128) * 128   # first query that sees this chunk
                if s0 >= sb_start + SBW:
                    continue                      # chunk entirely in the future
                n_off = max(0, s0 - sb_start)
                tri = s0 >= sb_start              # diagonal strip inside block
                pairs[sb].append((mc, n_off, tri, mode == "generic"))
            else:
                pairs[sb].append((mc, 0, False, mode == "generic"))
        # ensure first pair full width
        assert pairs[sb][0][1] == 0
    return pairs


def build_nc(mode, reps=1, phases=('proj', 'attn', 'oproj')):
    if mode != "causal":
        return _build_nc_ref(mode, reps, phases)
    nc = bacc.Bacc("TRN2", target_bir_lowering=False, debug=False, num_devices=8)

    # xt chunked for streaming: [b, sb, kg, 128, 4ko, 512s]
    xt = nc.dram_tensor("xt", [B, NSB, NKG, 128, 8, SBW], mybir.dt.bfloat16,
                        kind="ExternalInput")
    wq = nc.dram_tensor("wq", [128, QH, KCH, HD], mybir.dt.bfloat16, kind="ExternalInput")
    wk = nc.dram_tensor("wk", [128, KCH, HD], mybir.dt.bfloat16, kind="ExternalInput")
    wv = nc.dram_tensor("wv", [128, KCH, HD], mybir.dt.bfloat16, kind="ExternalInput")
    wo = nc.dram_tensor("wo", [128, QH, DIM], mybir.dt.bfloat16, kind="ExternalInput")
    kc = nc.dram_tensor("kc", [B, 128, CACHE], mybir.dt.bfloat16, kind="ExternalInput")
    vc = nc.dram_tensor("vc", [B, 128, CACHE], mybir.dt.bfloat16, kind="ExternalInput")
    cc = nc.dram_tensor("cc", [128, S], F32, kind="ExternalInput")
    ss = nc.dram_tensor("ss", [128, S], F32, kind="ExternalInput")
    tri = nc.dram_tensor("tri", [128, 128], F32, kind="ExternalInput")
    ident = nc.dram_tensor("ident", [128, 128], mybir.dt.bfloat16, kind="ExternalInput")
    out = nc.dram_tensor("o", [B * S, DIM], mybir.dt.bfloat16, kind="ExternalOutput")

    pairs = _pairs_for_mode(mode)

    with tile.TileContext(nc) as tc:
        with (
            tc.tile_pool(name="const", bufs=1) as const,
            tc.tile_pool(name="big", bufs=2) as big,
            tc.tile_pool(name="work", bufs=2) as work,
            tc.tile_pool(name="ps_mm", bufs=2, space="PSUM") as ps_mm,
            tc.tile_pool(name="ps_sc", bufs=3, space="PSUM") as ps_sc,
            tc.tile_pool(name="ps_av", bufs=2, space="PSUM") as ps_av,
        ):
          from contextlib import nullcontext
          _hints = (mybir.EngineType.PE, mybir.EngineType.DVE,
                    mybir.EngineType.Activation, mybir.EngineType.Pool,
                    mybir.EngineType.SP)
          with (tc.For_i(0, reps, 1, hint_engines=_hints) if reps > 1
                else nullcontext()):
            # ---- constants ----
            wk_sb = const.tile([128, KCH, HD], BF)
            nc.sync.dma_start(wk_sb, wk.ap())
            cc_sb = const.tile([128, S], F32)
            nc.sync.dma_start(cc_sb, cc.ap())
            ss_sb = const.tile([128, S], F32)
            nc.sync.dma_start(ss_sb, ss.ap())
            # deferred constants (DMA emitted inside kv_stream(0) at need)
            wv_sb = const.tile([128, KCH, HD], BF)
            tri_sb = const.tile([128, 128], F32)
            id_sb = const.tile([128, 128], BF)
            ones_bf = const.tile([128, 1], BF)
            nc.vector.memset(ones_bf, 1.0)
            ones_k1 = const.tile([1, 128], BF)
            nc.vector.memset(ones_k1, 1.0)

            def rope_store(ps, dest, scol):
                """dest[0:64] = ps[0:64]*cc - ps[64:]*ss ; dest[64:] = ps[0:64]*ss + ps[64:]*cc
                ps: [128, SBW] psum f32; dest: [128, SBW] sbuf bf16 slice; scol: s offset.
                Every tensor_tensor keeps equal input base partitions (walrus
                birverifier requirement); cross terms write to the other half."""
                cslc = cc_sb[:, scol:scol + SBW]
                sslc = ss_sb[:, scol:scol + SBW]
                t1 = work.tile([128, SBW], F32, name="rope_t1")
                nc.vector.tensor_mul(t1, ps, cslc)
                t2 = work.tile([128, SBW], F32, name="rope_t2")
                nc.vector.tensor_mul(t2[0:64, :], ps[64:128, :], sslc[64:128, :])
                nc.vector.tensor_mul(t2[64:128, :], ps[0:64, :], sslc[0:64, :])
                nc.vector.tensor_sub(dest[0:64, :], t1[0:64, :], t2[0:64, :])
                nc.vector.tensor_add(dest[64:128, :], t2[64:128, :], t1[64:128, :])

            # per-batch persistent tiles
            state = {}

            def batch_tiles(b):
                st = {}
                st['xt'] = {}
                st['kT'] = big.tile([128, KV], BF, name="kT")
                st['v'] = big.tile([128, NCH * 128], BF, name="v_sb")
                st['qT'] = big.tile([128, QH, S], BF, name="qT")
                st['outT'] = big.tile([128, QH, S], BF, name="outT")
                return st

            MMNS = 230      # ~ns per 512-wide matmul incl overhead

            def xsl(st, sb, ko):
                return st['xt'][(sb, ko // 8)][:, ko % 8, :]

            def kv_stream(b):
                """Setup + K and V projections. MUST be first-advanced before
                q_stream(b) (creates state[b])."""
                st = state[b] = batch_tiles(b)
                # first x chunks now; later chunks staggered into K-proj units
                def emit_chunk(kg):
                    for sb in range(NSB):
                        t = big.tile([128, 8, SBW], BF, name="xt_t", bufs=10)
                        nc.sync.dma_start(t, xt.ap()[b, sb, kg])
                        st['xt'][(sb, kg)] = t
                emit_chunk(0)
                yield 0

                # ---- K projection + RoPE ----
                for sb in range(NSB):
                    ps = ps_mm.tile([128, SBW], F32, name="mm_ps", bufs=1)
                    for ko in range(KCH):
                        nc.tensor.matmul(ps, wk_sb[:, ko, :], xsl(st, sb, ko),
                                         start=(ko == 0), stop=(ko == KCH - 1))
                        if ko % 2 == 1:
                            if sb == 0 and ko // 2 + 1 < NKG:
                                emit_chunk(ko // 2 + 1)
                            if sb == 0 and ko == 7 and b == 0:
                                nc.sync.dma_start(wv_sb, wv.ap())
                            if sb == 0 and ko == 9 and b == 0:
                                nc.sync.dma_start(id_sb, ident.ap())
                                nc.sync.dma_start(tri_sb, tri.ap())
                            yield MMNS * 2
                    rope_store(ps, st['kT'][:, CACHE + sb * SBW: CACHE + (sb + 1) * SBW],
                               sb * SBW)
                    yield 0

                # kv cache loads (needed only at attention)
                nc.sync.dma_start(st['kT'][:, 0:CACHE], kc.ap()[b])
                nc.sync.dma_start(st['v'][:, 0:CACHE], vc.ap()[b])

                # ---- V projection (transposed, then PE-transpose to natural) ----
                for sb in range(NSB):
                    ps = ps_mm.tile([128, SBW], F32, name="mm_ps", bufs=1)
                    for ko in range(KCH):
                        nc.tensor.matmul(ps, wv_sb[:, ko, :], xsl(st, sb, ko),
                                         start=(ko == 0), stop=(ko == KCH - 1))
                        if ko % 2 == 1:
                            yield MMNS * 2
                    vt = work.tile([128, SBW], BF, name="vT_sb")
                    nc.vector.tensor_copy(vt, ps)
                    yield 0
                    pst = ps_mm.tile([128, SBW], BF, name="mm_ps", bufs=1)
                    for j in range(4):
                        nc.tensor.transpose(
                            pst[:, j * 128:(j + 1) * 128],
                            vt[:, j * 128:(j + 1) * 128], id_sb)
                    yield MMNS
                    nc.vector.tensor_copy(
                        st['v'][:, CACHE + sb * SBW: CACHE + (sb + 1) * SBW], pst)
                    yield 0

            def q_stream(b):
                """Q projection + RoPE; advance only after kv_stream(b) started."""
                st = state[b]
                for h in range(QH):
                    wq_p = []
                    for pc in range(4):
                        t = work.tile([128, 8, HD], BF, name="wq_t", bufs=5)
                        nc.sync.dma_start(t, wq.ap()[:, h, pc * 8:(pc + 1) * 8, :])
                        wq_p.append(t)
                    for sb in range(NSB):
                        ps = ps_mm.tile([128, SBW], F32, name="mm_ps2")
                        for ko in range(KCH):
                            nc.tensor.matmul(ps, wq_p[ko // 8][:, ko % 8, :],
                                             xsl(st, sb, ko),
                                             start=(ko == 0), stop=(ko == KCH - 1))
                            if ko % 2 == 1:
                                yield MMNS * 2
                        rope_store(ps, st['qT'][:, h, sb * SBW:(sb + 1) * SBW],
                                   sb * SBW)
                        yield 0

            def merged(*gens):
                """Time-balanced interleave; first generator is advanced first
                (ordering matters when later gens read its setup state)."""
                n = len(gens)
                acc = [0.0] * n
                cur = [next(g, None) for g in gens]
                while any(c is not None for c in cur):
                    i = min((a for a in range(n) if cur[a] is not None),
                            key=lambda a: acc[a])
                    acc[i] += cur[i]
                    yield cur[i]
                    cur[i] = next(gens[i], None)

            def proj_stream(b):
                """kv and q interleaved (startup overlap within a batch)."""
                return merged(kv_stream(b), q_stream(b))

            def attn_stream(b):
                st = state[b]
                ep = None       # pending epilogue closure chain
                for sb in range(NSB):
                    plist = pairs[sb]
                    for h in range(QH):
                        # two parallel softmax-sum chains: DVE gets every 3rd
                        # pair, GpSimd the rest (GpSimd op ~2x slower)
                        acc_a = work.tile([128, SBW], BF, name="acc_a", bufs=2)
                        acc_b = work.tile([128, SBW], BF, name="acc_b", bufs=2)
                        na = nb_ = 0
                        av = ps_av.tile([128, SBW], F32, name="av_ps")
                        pend = []
                        av_n = 0
                        for pi, (mc, n_off, use_tri, _) in enumerate(plist):
                            sc = ps_sc.tile([128, SBW], F32, name="sc_ps")
                            nc.tensor.matmul(
                                sc[:, n_off:], st['kT'][:, mc * 128:(mc + 1) * 128],
                                st['qT'][:, h, sb * SBW + n_off:(sb + 1) * SBW],
                                start=True, stop=True)
                            if use_tri:
                                nc.vector.tensor_add(
                                    sc[:, n_off:n_off + 128],
                                    sc[:, n_off:n_off + 128], tri_sb)
                            ex = work.tile([128, SBW], BF, name="ex", bufs=5)
                            nc.scalar.activation(ex[:, n_off:], sc[:, n_off:],
                                                 mybir.ActivationFunctionType.Exp,
                                                 scale=SCALE)
                            if pi % 3 == 0:
                                if na == 0:
                                    nc.vector.tensor_copy(acc_a, ex)
                                else:
                                    nc.vector.tensor_add(acc_a[:, n_off:],
                                                         acc_a[:, n_off:],
                                                         ex[:, n_off:])
                                na += 1
                            else:
                                if nb_ == 0:
                                    nc.gpsimd.tensor_copy(acc_b, ex)
                                else:
                                    nc.gpsimd.tensor_add(acc_b[:, n_off:],
                                                         acc_b[:, n_off:],
                                                         ex[:, n_off:])
                                nb_ += 1
                            pend.append((mc, n_off, ex))
                            if len(pend) > 2:
                                pmc, poff, pex = pend.pop(0)
                                nc.tensor.matmul(av[:, poff:],
                                                 st['v'][:, pmc * 128:(pmc + 1) * 128],
                                                 pex[:, poff:],
                                                 start=(av_n == 0), stop=False)
                                av_n += 1
                            if ep is not None:
                                ep = ep()   # run one lagged epilogue step
                            yield MMNS * 2
                        # flush remaining avs; denominator lagged into next block
                        for j, (pmc, poff, pex) in enumerate(pend):
                            nc.tensor.matmul(av[:, poff:],
                                             st['v'][:, pmc * 128:(pmc + 1) * 128],
                                             pex[:, poff:],
                                             start=(av_n == 0),
                                             stop=(j == len(pend) - 1))
                            av_n += 1
                            if ep is not None:
                                ep = ep()
                            yield MMNS
                        while ep is not None:
                            ep = ep()

                        def make_ep(acc_a=acc_a, acc_b=acc_b, av=av, h=h, sb=sb):
                            def e0():
                                den = ps_sc.tile([128, SBW], F32, name="sc_ps")
                                nc.tensor.matmul(den[0:1, :], ones_bf, acc_a,
                                                 start=True, stop=False)
                                nc.tensor.matmul(den[0:1, :], ones_bf, acc_b,
                                                 start=False, stop=True)

                                def e1():
                                    rc = work.tile([1, SBW], F32, name="rc", bufs=2)
                                    nc.vector.reciprocal_approx_fast(rc, den[0:1, :])
                                    rcb = work.tile([1, SBW], BF, name="rcb", bufs=2)
                                    nc.scalar.copy(rcb, rc)

                                    def e2():
                                        rb = ps_sc.tile([128, SBW], F32, name="sc_ps")
                                        nc.tensor.matmul(rb, ones_k1, rcb,
                                                         start=True, stop=True)
                                        rbs = work.tile([128, SBW], F32, name="rbs",
                                                        bufs=2)
                                        nc.vector.tensor_copy(rbs, rb)

                                        def e3():
                                            nc.vector.tensor_mul(
                                                st['outT'][:, h,
                                                           sb * SBW:(sb + 1) * SBW],
                                                av, rbs)
                                            return None
                                        return e3
                                    return e2
                                return e1
                            return e0
                        ep = make_ep()
                while ep is not None:
                    ep = ep()
                yield 0

            def oproj_stream(b, act_mod=5, extra_pools=False):
                st = state[b]
                ui = 0
                pool_cycle = ([(ps_mm, "mm_ps2"), (ps_sc, "sc_ps"),
                               (ps_av, "av_ps")] if extra_pools
                              else [(ps_mm, "mm_ps2")])
                for nb in range(DIM // SBW):
                    wo_t = work.tile([128, QH, SBW], BF, name="wo_t", bufs=2)
                    nc.sync.dma_start(wo_t[:, 0:2], wo.ap()[:, 0:2, nb * SBW:(nb + 1) * SBW])
                    nc.sync.dma_start(wo_t[:, 2:4], wo.ap()[:, 2:4, nb * SBW:(nb + 1) * SBW])
                    for stt in range(S // 128):
                        pool, tag = pool_cycle[ui % len(pool_cycle)]
                        ps = pool.tile([128, SBW], F32, name=tag)
                        for h in range(QH):
                            nc.tensor.matmul(ps, st['outT'][:, h, stt * 128:(stt + 1) * 128],
                                             wo_t[:, h, :],
                                             start=(h == 0), stop=(h == QH - 1))
                        os_t = work.tile([128, SBW], BF, name="os_t", bufs=4)
                        if ui % act_mod == act_mod - 1:
                            nc.scalar.copy(os_t, ps)
                        else:
                            nc.vector.tensor_copy(os_t, ps)
                        nc.sync.dma_start(
                            out.ap()[b * S + stt * 128:b * S + (stt + 1) * 128,
                                     nb * SBW:(nb + 1) * SBW], os_t)
                        ui += 1
                        yield MMNS * 4

            def drain(g):
                for _ in g:
                    pass

            def merge2(ga, gb):
                ta = tb = 0.0
                ca = next(ga, None)
                cb = next(gb, None)
                while ca is not None or cb is not None:
                    if cb is None or (ca is not None and ta <= tb):
                        ta += ca
                        ca = next(ga, None)
                    else:
                        tb += cb
                        cb = next(gb, None)

            drain(proj_stream(0))                        # stage 1
            merge2(attn_stream(0), proj_stream(1))       # stage 2
            merge2(oproj_stream(0, 5), attn_stream(1))   # stage 3
            drain(oproj_stream(1, 2, extra_pools=True))  # stage 4

    nc.compile()
    return nc


def _build_nc_ref(mode, reps=1, phases=('proj', 'attn', 'oproj')):
    nc = bacc.Bacc("TRN2", target_bir_lowering=False, debug=False, num_devices=8)

    xt = nc.dram_tensor("xt", [128, KCH, B * S], mybir.dt.bfloat16, kind="ExternalInput")
    wq = nc.dram_tensor("wq", [128, QH, KCH, HD], mybir.dt.bfloat16, kind="ExternalInput")
    wk = nc.dram_tensor("wk", [128, KCH, HD], mybir.dt.bfloat16, kind="ExternalInput")
    wv = nc.dram_tensor("wv", [128, KCH, HD], mybir.dt.bfloat16, kind="ExternalInput")
    wo = nc.dram_tensor("wo", [128, QH, DIM], mybir.dt.bfloat16, kind="ExternalInput")
    kc = nc.dram_tensor("kc", [B, 128, CACHE], mybir.dt.bfloat16, kind="ExternalInput")
    vc = nc.dram_tensor("vc", [B, 128, CACHE], mybir.dt.bfloat16, kind="ExternalInput")
    cc = nc.dram_tensor("cc", [128, S], F32, kind="ExternalInput")
    ss = nc.dram_tensor("ss", [128, S], F32, kind="ExternalInput")
    tri = nc.dram_tensor("tri", [128, 128], F32, kind="ExternalInput")
    if mode == "generic":
        mt = nc.dram_tensor("maskt", [128, NCH, S], mybir.dt.bfloat16, kind="ExternalInput")
    out = nc.dram_tensor("o", [B * S, DIM], mybir.dt.bfloat16, kind="ExternalOutput")

    pairs = _pairs_for_mode(mode)

    with tile.TileContext(nc) as tc:
        with (
            tc.tile_pool(name="const", bufs=1) as const,
            tc.tile_pool(name="xpool", bufs=1) as xpool,
            tc.tile_pool(name="perb", bufs=2) as perb,
            tc.tile_pool(name="work", bufs=2) as work,
            tc.tile_pool(name="ps_mm", bufs=3, space="PSUM") as ps_mm,
            tc.tile_pool(name="ps_av", bufs=2, space="PSUM") as ps_av,
            tc.tile_pool(name="ps_sm", bufs=1, space="PSUM") as ps_sm,
        ):
          from contextlib import nullcontext
          with (tc.For_i(0, reps, 1) if reps > 1 else nullcontext()):
            # ---- constants ----
            wk_sb = const.tile([128, KCH, HD], BF)
            nc.sync.dma_start(wk_sb, wk.ap())
            wv_sb = const.tile([128, KCH, HD], BF)
            nc.sync.dma_start(wv_sb, wv.ap())
            cc_sb = const.tile([128, S], F32)
            nc.sync.dma_start(cc_sb, cc.ap())
            ss_sb = const.tile([128, S], F32)
            nc.sync.dma_start(ss_sb, ss.ap())
            tri_sb = const.tile([128, 128], F32)
            nc.sync.dma_start(tri_sb, tri.ap())
            ones_bf = const.tile([128, 1], BF)
            nc.vector.memset(ones_bf, 1.0)
            ones_k1 = const.tile([1, 128], BF)
            nc.vector.memset(ones_k1, 1.0)

            def rope_store(ps, dest, scol):
                cslc = cc_sb[:, scol:scol + SBW]
                sslc = ss_sb[:, scol:scol + SBW]
                t1 = work.tile([128, SBW], F32, name="rope_t1")
                nc.vector.tensor_mul(t1, ps, cslc)
                t2 = work.tile([128, SBW], F32, name="rope_t2")
                nc.vector.tensor_mul(t2[0:64, :], ps[64:128, :], sslc[64:128, :])
                nc.vector.tensor_mul(t2[64:128, :], ps[0:64, :], sslc[0:64, :])
                nc.vector.tensor_sub(dest[0:64, :], t1[0:64, :], t2[0:64, :])
                nc.vector.tensor_add(dest[64:128, :], t2[64:128, :], t1[64:128, :])

            for b in range(B):
                xT = xpool.tile([128, KCH, S], BF, name="xT")
                for g in range(4):
                    for hh in range(2):
                        nc.sync.dma_start(
                            xT[:, hh * 16:(hh + 1) * 16, g * 256:(g + 1) * 256],
                            xt.ap()[:, hh * 16:(hh + 1) * 16,
                                    b * S + g * 256: b * S + (g + 1) * 256])
                kT = perb.tile([128, KV], BF, name="kT")
                nc.sync.dma_start(kT[:, 0:CACHE], kc.ap()[b])
                v_sb = perb.tile([128, NCH, 128], BF, name="v_sb")
                nc.sync.dma_start(
                    v_sb[:, 0:CACHE // 128, :],
                    vc.ap()[b].rearrange("p (c j) -> p c j", j=128))

                for sb in (range(NSB) if 'proj' in phases else []):
                    ps = ps_mm.tile([128, SBW], F32, name="mm_ps")
                    for ko in range(KCH):
                        nc.tensor.matmul(ps, wk_sb[:, ko, :],
                                         xT[:, ko, sb * SBW:(sb + 1) * SBW],
                                         start=(ko == 0), stop=(ko == KCH - 1))
                    rope_store(ps, kT[:, CACHE + sb * SBW: CACHE + (sb + 1) * SBW],
                               sb * SBW)

                qT = perb.tile([128, QH, S], BF, name="qT")
                for h in (range(QH) if 'proj' in phases else []):
                    wq_blk = work.tile([128, KCH, HD], BF, name="wq_blk")
                    nc.sync.dma_start(wq_blk, wq.ap()[:, h])
                    for sb in range(NSB):
                        ps = ps_mm.tile([128, SBW], F32, name="mm_ps")
                        for ko in range(KCH):
                            nc.tensor.matmul(ps, wq_blk[:, ko, :],
                                             xT[:, ko, sb * SBW:(sb + 1) * SBW],
                                             start=(ko == 0), stop=(ko == KCH - 1))
                        rope_store(ps, qT[:, h, sb * SBW:(sb + 1) * SBW], sb * SBW)

                for mt_i in (range(S // 128) if 'proj' in phases else []):
                    ps = ps_mm.tile([128, SBW], F32, name="mm_ps")
                    for ko in range(KCH):
                        nc.tensor.matmul(
                            ps[:, 0:128],
                            xT[:, ko, mt_i * 128:(mt_i + 1) * 128],
                            wv_sb[:, ko, :],
                            start=(ko == 0), stop=(ko == KCH - 1))
                    if mt_i % 2 == 0:
                        nc.scalar.copy(v_sb[:, CACHE // 128 + mt_i, :], ps[:, 0:128])
                    else:
                        nc.vector.tensor_copy(v_sb[:, CACHE // 128 + mt_i, :], ps[:, 0:128])

                outT = None
                if 'attn' in phases or 'oproj' in phases:
                    outT = perb.tile([128, QH, S], BF, name="outT", bufs=1)
                    if 'attn' not in phases:
                        nc.vector.memset(outT, 0.0)
                for h in (range(QH) if 'attn' in phases else []):
                    for sb in range(NSB):
                        av = ps_av.tile([128, SBW], F32, name="av_ps")
                        acc = work.tile([128, SBW], BF, name="acc", bufs=3)
                        plist = pairs[sb]
                        for pi, (mc, n_off, use_tri, use_mask) in enumerate(plist):
                            w = SBW - n_off
                            sc = ps_mm.tile([128, SBW], F32, name="mm_ps")
                            nc.tensor.matmul(
                                sc[:, n_off:], kT[:, mc * 128:(mc + 1) * 128],
                                qT[:, h, sb * SBW + n_off:(sb + 1) * SBW],
                                start=True, stop=True)
                            if use_tri:
                                nc.vector.tensor_add(
                                    sc[:, n_off:n_off + 128],
                                    sc[:, n_off:n_off + 128], tri_sb)
                            if use_mask:
                                mblk = work.tile([128, SBW], BF, name="mblk", bufs=3)
                                nc.sync.dma_start(
                                    mblk, mt.ap()[:, mc, sb * SBW:(sb + 1) * SBW])
                                nc.vector.tensor_add(sc[:, n_off:], sc[:, n_off:],
                                                     mblk[:, n_off:])
                            ex = work.tile([128, SBW], BF, name="ex", bufs=6)
                            nc.scalar.activation(ex[:, n_off:], sc[:, n_off:],
                                                 mybir.ActivationFunctionType.Exp,
                                                 scale=SCALE)
                            nc.tensor.matmul(av[:, n_off:], v_sb[:, mc, :],
                                             ex[:, n_off:],
                                             start=(pi == 0), stop=(pi == len(plist) - 1))
                            if pi == 0:
                                nc.vector.tensor_copy(acc, ex)
                            else:
                                nc.vector.tensor_add(acc[:, n_off:], acc[:, n_off:],
                                                     ex[:, n_off:])
                        den = ps_sm.tile([1, SBW], F32, name="den_ps", bufs=2)
                        nc.tensor.matmul(den, ones_bf, acc, start=True, stop=True)
                        rc = work.tile([1, SBW], BF, name="rc")
                        with nc.allow_low_precision(reason="1/denom in bf16 feeds bf16 matmuls"):
                            nc.vector.reciprocal(rc, den)
                        rb = ps_sm.tile([128, SBW], F32, name="rb_ps")
                        nc.tensor.matmul(rb, ones_k1, rc, start=True, stop=True)
                        rbs = work.tile([128, SBW], F32, name="rbs")
                        nc.scalar.copy(rbs, rb)
                        nc.vector.tensor_mul(outT[:, h, sb * SBW:(sb + 1) * SBW],
                                             av, rbs)

                for nb in (range(DIM // SBW) if 'oproj' in phases else []):
                    wo_blk = work.tile([128, QH, SBW], BF, name="wo_blk", bufs=3)
                    nc.sync.dma_start(wo_blk[:, 0:2], wo.ap()[:, 0:2, nb * SBW:(nb + 1) * SBW])
                    nc.sync.dma_start(wo_blk[:, 2:4], wo.ap()[:, 2:4, nb * SBW:(nb + 1) * SBW])
                    for st in range(S // 128):
                        ps = ps_mm.tile([128, SBW], F32, name="mm_ps")
                        for h in range(QH):
                            nc.tensor.matmul(ps, outT[:, h, st * 128:(st + 1) * 128],
                                             wo_blk[:, h, :],
                                             start=(h == 0), stop=(h == QH - 1))
                        os_t = work.tile([128, SBW], BF, name="os_t", bufs=6)
                        if st % 2 == 0:
                            nc.scalar.copy(os_t, ps)
                        else:
                            nc.vector.tensor_copy(os_t, ps)
                        nc.sync.dma_start(
                            out.ap()[b * S + st * 128:b * S + (st + 1) * 128,
                                     nb * SBW:(nb + 1) * SBW], os_t)

    nc.compile()
    return nc


def _detect_mode(mask):
    m = np.asarray(mask).reshape(S, KV)
    if not np.all(m[:, :CACHE] == 0):
        return "generic"
    new = m[:, CACHE:]
    neg = np.min(new)
    if neg == 0:
        return "full" if np.all(new == 0) else "generic"
    causal = np.triu(np.full((S, S), neg, dtype=np.float32), k=1)
    if np.array_equal(new, causal):
        return "causal"
    return "generic"


def prepare_inputs(x, freqs_cos, freqs_sin, mask, cache_k, cache_v, wq, wk, wv, wo,
                   mode):
    """Build the 8 per-core in_maps (numpy, host-side sharding + layout)."""
    perm = np.concatenate([np.arange(0, HD, 2), np.arange(1, HD, 2)])
    x2 = np.asarray(x, np.float32).reshape(B * S, DIM)
    xt_flat = np.ascontiguousarray(
        x2.reshape(B * S, KCH, 128).transpose(2, 1, 0)).astype(BF16)  # [128,KCH,BS]
    if mode == "causal":
        # [b, sb, kg, 128, 4, SBW]
        xt = np.ascontiguousarray(
            xt_flat.reshape(128, NKG, 8, B, NSB, SBW).transpose(3, 4, 1, 0, 2, 5))
    else:
        xt = xt_flat
    cos = np.asarray(freqs_cos, np.float32)
    sin = np.asarray(freqs_sin, np.float32)
    cc = np.ascontiguousarray(np.vstack([cos.T, cos.T]))  # [128, S]
    ss = np.ascontiguousarray(np.vstack([sin.T, sin.T]))
    m0 = np.asarray(mask, np.float32).reshape(S, KV)
    inv_scale = 1.0 / SCALE
    if mode == "causal":
        tri_np = np.ascontiguousarray(m0[0:128, CACHE:CACHE + 128].T) * inv_scale
    else:
        tri_np = np.zeros((128, 128), np.float32)
    ident_np = np.eye(128, dtype=BF16)
    maskt = None
    if mode == "generic":
        maskt = np.ascontiguousarray(
            (m0.T * inv_scale).reshape(NCH, 128, S).transpose(1, 0, 2)).astype(BF16)

    wq_f = np.asarray(wq, np.float32)
    wk_f = np.asarray(wk, np.float32)
    wv_f = np.asarray(wv, np.float32)
    wo_f = np.asarray(wo, np.float32)
    ck = np.asarray(cache_k, np.float32)
    cv = np.asarray(cache_v, np.float32)

    in_maps = []
    for c in range(8):
        wq_c = wq_f[:, c * 512:(c + 1) * 512].reshape(DIM, QH, HD)[:, :, perm]
        wq_r = np.ascontiguousarray(
            wq_c.reshape(KCH, 128, QH, HD).transpose(1, 2, 0, 3)).astype(BF16)
        wk_c = wk_f[:, c * 128:(c + 1) * 128][:, perm]
        wk_r = np.ascontiguousarray(
            wk_c.reshape(KCH, 128, HD).transpose(1, 0, 2)).astype(BF16)
        wv_c = wv_f[:, c * 128:(c + 1) * 128]
        wv_r = np.ascontiguousarray(
            wv_c.reshape(KCH, 128, HD).transpose(1, 0, 2)).astype(BF16)
        wo_c = wo_f[c * 512:(c + 1) * 512, :]
        wo_r = np.ascontiguousarray(
            wo_c.reshape(QH, 128, DIM).transpose(1, 0, 2)).astype(BF16)
        kc_r = np.ascontiguousarray(
            ck[:, c][:, :, perm].transpose(0, 2, 1)).astype(BF16)  # [B,128,CACHE]
        vc_r = np.ascontiguousarray(
            cv[:, c].reshape(B, CACHE // 128, 128, HD).transpose(0, 2, 1, 3)
            .reshape(B, 128, CACHE)).astype(BF16)
        im = {"xt": xt, "wq": wq_r, "wk": wk_r, "wv": wv_r, "wo": wo_r,
              "kc": kc_r, "vc": vc_r, "cc": cc, "ss": ss, "tri": tri_np}
        if mode == "causal":
            im["ident"] = ident_np
        if maskt is not None:
            im["maskt"] = maskt
        in_maps.append(im)
    return in_maps


def kernel(x, freqs_cos, freqs_sin, mask, cache_k, cache_v, wq, wk, wv, wo):
    mode = _detect_mode(mask)
    in_maps = prepare_inputs(x, freqs_cos, freqs_sin, mask, cache_k, cache_v,
                             wq, wk, wv, wo, mode)
    if mode not in _CACHE:
        _CACHE[mode] = build_nc(mode)
    nc = _CACHE[mode]
    res = run_bass_kernel_spmd(nc, in_maps, core_ids=list(range(8)))
    total = res.results[0]["o"].astype(np.float32)
    for c in range(1, 8):
        total += res.results[c]["o"].astype(np.float32)
    return total.reshape(B, S, DIM)
